# revision 4
# baseline (speedup 1.0000x reference)
"""PointPillars Trainium2 kernel v2.

Pipeline per core (8 cores, 64-row horizontal bands of the 512x512 canvas):
  1. PFN: each matmul column holds one point of an even-x pillar (K rows
     0:8) AND one point of an odd-x pillar (rows 8:16) via block-diagonal
     weights (K=16, M=128 = even-pillar 64ch | odd-pillar 64ch).  Pillars
     are paired rank-wise by valid-point count within each row-quad;
     chunks of 32 pair-slots with data-dependent point depth (sorted,
     cross-core max, +1 all-zero column for built-in relu).  TensorE
     matmul -> PSUM, DVE tensor_reduce (max over points) -> emb, already
     in the x-parity-split partition layout the scatter needs.
  2. Scatter: gpsimd local_scatter writes embeddings straight into an
     x-parity-deinterleaved BEV canvas (partitions 0:64 = even x, 64:128 =
     odd x), zeroing untouched cells.  Host precomputes int16 indices.
  3. conv1 64->128 s2: x-parity pairing -> 6 matmul passes/chunk instead of
     9 (3 taps K=128 + 3 taps K=64), tap-major in groups of 4 psum banks.
  4. conv2 128->128 s1: 9 taps K=128 tap-major, groups of 4.
  5. head 1x1 -> 34ch, bias on ScalarE, DMA out.
"""

import os
import sys
from contextlib import ExitStack

sys.path.insert(0, "/opt/trn_rl_repo")

import numpy as np
import ml_dtypes

import concourse.bass as bass
import concourse.tile as tile
from concourse import bacc, mybir
from concourse.bass_utils import run_bass_kernel_spmd

BF16 = mybir.dt.bfloat16
F32 = mybir.dt.float32
I16 = mybir.dt.int16
NPBF16 = ml_dtypes.bfloat16

NCORES = 8
H = W = 512
P = 32
CE = 64
ROWS = 72          # rows per core band: 2 top halo + 64 owned + 6 bottom
NQ = ROWS // 4     # 18 row-quads
QSLOTS = 128       # pillar-pair slots per quad
NSLOTS = NQ * QSLOTS
SCH = 32           # pair-slots per PFN chunk
CANW = 257         # canvas cols: xe 0..255 data + col 256 zero
C1R = 34
OUTR = 32


# ----------------------------------------------------------------------------
# device program
# ----------------------------------------------------------------------------

def _build_program(profile, tot):
    """profile: tuple of (q, c, D, off) with D = pair depth (incl. zero pair),
    off = column offset of the chunk in x16."""
    nc = bacc.Bacc(None, target_bir_lowering=False, debug=False)

    x16 = nc.dram_tensor("x16", [16, tot], BF16, kind="ExternalInput")
    idxv = nc.dram_tensor("idxv", [128, NSLOTS], I16, kind="ExternalInput")
    w16 = nc.dram_tensor("w16", [16, 128], BF16, kind="ExternalInput")
    wp = nc.dram_tensor("wp", [128, 3 * 128], BF16, kind="ExternalInput")
    ws = nc.dram_tensor("ws", [64, 3 * 128], BF16, kind="ExternalInput")
    wc2 = nc.dram_tensor("wc2", [128, 9 * 128], BF16, kind="ExternalInput")
    whd = nc.dram_tensor("whd", [128, 34], BF16, kind="ExternalInput")
    b1v = nc.dram_tensor("b1v", [128, 1], F32, kind="ExternalInput")
    b2v = nc.dram_tensor("b2v", [128, 1], F32, kind="ExternalInput")
    bhd = nc.dram_tensor("bhd", [34, 1], F32, kind="ExternalInput")
    rmask = nc.dram_tensor("rmask", [128, 2], F32, kind="ExternalInput")
    out = nc.dram_tensor("out", [34, OUTR, 256], F32, kind="ExternalOutput")

    by_quad = {}
    for (q, c, d, off) in profile:
        by_quad.setdefault(q, []).append((c, d, off))

    with tile.TileContext(nc) as tc, ExitStack() as ctx:
        const = ctx.enter_context(tc.tile_pool(name="const", bufs=1))
        big = ctx.enter_context(tc.tile_pool(name="big", bufs=1))
        stg = ctx.enter_context(tc.tile_pool(name="staging", bufs=4))
        ps = ctx.enter_context(tc.tile_pool(name="psum", bufs=2, space="PSUM"))

        # ---- DMAs ordered for earliest compute start, split over 2 queues.
        # scalar queue leads with PFN weights; sync queue streams x16 slices.
        w16_sb = const.tile([16, 128], BF16)
        nc.scalar.dma_start(w16_sb[:], w16[:])
        x16_sb = big.tile([16, tot], BF16)
        qoff = [by_quad[q][0][2] if q in by_quad else None for q in range(NQ)]
        qoff.append(tot)
        for k in range(NQ - 1, -1, -1):
            if qoff[k] is None:
                qoff[k] = qoff[k + 1]
        bnds = sorted(set([qoff[0], qoff[3], qoff[6], qoff[9], qoff[12], qoff[15], tot]))
        for a, b in zip(bnds[:-1], bnds[1:]):
            if b > a:
                nc.sync.dma_start(x16_sb[:, a:b], x16[:, a:b])
        # scalar queue: scatter indices + conv weights in consumption order.
        idx_sb = big.tile([128, NSLOTS], I16)
        nc.scalar.dma_start(idx_sb[:], idxv[:])
        wp_sb = const.tile([128, 3 * 128], BF16)
        nc.scalar.dma_start(wp_sb[:], wp[:])
        ws_sb = const.tile([64, 3 * 128], BF16)
        nc.scalar.dma_start(ws_sb[:], ws[:])
        b1_sb = const.tile([128, 1], F32)
        nc.scalar.dma_start(b1_sb[:], b1v[:])
        rmask_sb = const.tile([128, 2], F32)
        nc.scalar.dma_start(rmask_sb[:], rmask[:])
        wc2_sb = const.tile([128, 9 * 128], BF16)
        nc.scalar.dma_start(wc2_sb[:], wc2[:])
        b2_sb = const.tile([128, 1], F32)
        nc.scalar.dma_start(b2_sb[:], b2v[:])
        whd_sb = const.tile([128, 34], BF16)
        nc.scalar.dma_start(whd_sb[:], whd[:])
        bhd_sb = const.tile([34, 1], F32)
        nc.scalar.dma_start(bhd_sb[:], bhd[:])

        emb2 = big.tile([128, NSLOTS], BF16)
        canvas2 = big.tile([128, ROWS, CANW], BF16)
        out1 = big.tile([128, C1R, 258], BF16)
        out2 = big.tile([128, OUTR, 256], BF16)

        nc.vector.memset(out1[:, :, 0:1], 0.0)
        nc.vector.memset(out1[:, :, 257:258], 0.0)

        def pfn_quad(q):
            for (c, d, off) in by_quad.get(q, []):
                cols = SCH * d
                pt = ps.tile([128, 1024], F32, tag="pfn", bufs=2, name=f"pt{q}_{c}")
                for a in range(0, cols, 512):
                    b = min(a + 512, cols)
                    nc.tensor.matmul(
                        pt[:, a:b], lhsT=w16_sb[:], rhs=x16_sb[:, off + a : off + b],
                        start=True, stop=True,
                    )
                s0 = q * QSLOTS + c * SCH
                nc.vector.tensor_reduce(
                    emb2[:, s0 : s0 + SCH],
                    pt[:, 0:cols].rearrange("p (s d) -> p s d", s=SCH),
                    axis=mybir.AxisListType.X,
                    op=mybir.AluOpType.max,
                )
            qb = q * QSLOTS
            nc.gpsimd.local_scatter(
                canvas2[:, 4 * q : 4 * q + 4, :],
                emb2[:, qb : qb + QSLOTS],
                idx_sb[:, qb : qb + QSLOTS],
                channels=128,
                num_elems=4 * CANW,
                num_idxs=QSLOTS,
            )

        def conv1_chunk(c):
            pc = ps.tile([128, 2, 256], F32, tag="mm", bufs=4, name=f"pc1_{c}")
            for ki in range(6):
                dy, paired = ki % 3, ki < 3
                r = 4 * c + dy
                if paired:
                    rhs = canvas2[:, r : r + 3 : 2, 0:256]
                    lhsT = wp_sb[:, dy * 128 : (dy + 1) * 128]
                else:
                    rhs = canvas2[0:64, r : r + 3 : 2, 1:257]
                    lhsT = ws_sb[:, dy * 128 : (dy + 1) * 128]
                nc.tensor.matmul(
                    pc[:], lhsT=lhsT, rhs=rhs, start=(ki == 0), stop=(ki == 5)
                )
            nc.scalar.activation(
                out1[:, 2 * c : 2 * c + 2, 1:257], pc[:],
                mybir.ActivationFunctionType.Relu, bias=b1_sb[:],
            )
            # zero conv1 halo rows that are conv2 SAME-padding at global edges
            if c == 0:
                nc.vector.tensor_scalar_mul(
                    out1[:, 0:1, :], out1[:, 0:1, :], rmask_sb[:, 0:1])
            if c == C1R // 2 - 1:
                nc.vector.tensor_scalar_mul(
                    out1[:, 33:34, :], out1[:, 33:34, :], rmask_sb[:, 1:2])

        def conv2_chunk(c):
            o = 2 * c
            pc = ps.tile([128, 2, 256], F32, tag="mm", bufs=4, name=f"pc2_{c}")
            for k in range(9):
                dy, dx = k // 3, k % 3
                rhs = out1[:, o + dy : o + dy + 2, dx : dx + 256]
                nc.tensor.matmul(
                    pc[:], lhsT=wc2_sb[:, 128 * k : 128 * (k + 1)], rhs=rhs,
                    start=(k == 0), stop=(k == 8),
                )
            nc.scalar.activation(
                out2[:, o : o + 2, :], pc[:],
                mybir.ActivationFunctionType.Relu, bias=b2_sb[:],
            )

        def head_chunk(c):
            o = 2 * c
            ph = ps.tile([34, 2, 256], F32, tag="mm", bufs=4, name=f"ph_{c}")
            nc.tensor.matmul(
                ph[:], lhsT=whd_sb[:], rhs=out2[:, o : o + 2, :],
                start=True, stop=True,
            )
            hstage = stg.tile([34, 2, 256], F32, tag="hstage", name=f"hs_{c}")
            nc.scalar.activation(
                hstage[:], ph[:], mybir.ActivationFunctionType.Identity,
                bias=bhd_sb[:],
            )
            eng = nc.sync if c % 2 == 0 else nc.scalar
            eng.dma_start(out[:, o : o + 2, :], hstage[:])

        # ---- fused quad pipeline with lag-2 so conv1 never waits on the
        # scatter of its own quad pair: PFN(s) | conv1(s-2) | conv2(s-3) |
        # head(s-4), plus drain steps.
        for s in range(NQ + 2):
            if s < NQ:
                pfn_quad(s)
            if 0 <= s - 2 < C1R // 2:
                conv1_chunk(s - 2)
            if 0 <= s - 3 < OUTR // 2:
                conv2_chunk(s - 3)
            if 0 <= s - 4 < OUTR // 2:
                head_chunk(s - 4)

    nc.compile()
    return nc


# ----------------------------------------------------------------------------
# host-side prep
# ----------------------------------------------------------------------------

def _prep(pillar_features, mask, coords, w_pfn, b_pfn,
          w1, b1, w2, b2, w_cls, b_cls, w_box, b_box):
    pf = np.asarray(pillar_features, np.float32)
    mk = np.asarray(mask, bool)
    xy = np.asarray(coords)
    x, y = xy[:, 0].astype(np.int64), xy[:, 1].astype(np.int64)

    valid = (x >= 0) & (x < W) & (y >= 0) & (y < H)
    lin = y * W + x
    # last-wins dedup among valid pillars (matches XLA scatter .set order)
    vidx = np.nonzero(valid)[0]
    order = vidx[np.argsort(lin[vidx], kind="stable")]
    ls = lin[order]
    is_last = np.ones(len(order), bool)
    if len(order) > 1:
        is_last[:-1] = ls[1:] != ls[:-1]
    keep = order[is_last]
    keep = keep[mk[keep].any(1)]  # empty pillars scatter zeros = canvas default

    kx, ky = x[keep], y[keep]
    kmask = mk[keep]
    vcnt = kmask.sum(1)

    # per (core, quad): pair even-x and odd-x pillars rank-wise by valid count
    cores = []
    for i in range(NCORES):
        y0 = 64 * i - 2
        sel = np.nonzero((ky >= y0) & (ky < y0 + ROWS))[0]
        rloc = (ky[sel] - y0).astype(np.int64)
        q = rloc // 4
        d = vcnt[sel]
        xpar = (kx[sel] % 2).astype(np.int64)
        slot = np.zeros(len(sel), np.int64)
        for t in range(NQ):
            for par in (0, 1):
                m = np.nonzero((q == t) & (xpar == par))[0]
                if len(m) > QSLOTS:
                    raise RuntimeError(f"quad overflow: {len(m)} > {QSLOTS}")
                o2 = m[np.argsort(-d[m], kind="stable")]
                slot[o2] = t * QSLOTS + np.arange(len(o2))
        cores.append({
            "keep": keep[sel], "rloc": rloc, "x": kx[sel],
            "depth": d, "slot": slot, "xpar": xpar,
        })

    # chunk profile: cross-core max point depth per (quad, chunk), +1 zero col
    prof = []
    off = 0
    for t in range(NQ):
        for c in range(QSLOTS // SCH):
            cid = t * (QSLOTS // SCH) + c
            dmax = 0
            for co in cores:
                m = (co["slot"] // SCH) == cid
                if m.any():
                    dmax = max(dmax, int(co["depth"][m].max()))
            if dmax == 0:
                continue
            d = dmax + 1
            prof.append((t, c, d, off))
            off += SCH * d
    tot = off
    profile = tuple(prof)

    # per-slot column base from profile
    colbase = np.full(NSLOTS, -1, np.int64)
    for (t, c, d, o) in prof:
        s0 = t * QSLOTS + c * SCH
        colbase[s0 : s0 + SCH] = o + np.arange(SCH) * d

    in_maps = []
    for i, co in enumerate(cores):
        x16t = np.zeros((tot, 16), np.float32)
        idx_even = np.full(NSLOTS, -1, np.int64)
        idx_odd = np.full(NSLOTS, -1, np.int64)

        kp, slot, xpar = co["keep"], co["slot"], co["xpar"]
        if len(kp):
            # target canvas index: (rloc%4)*CANW + xe
            tgt = (co["rloc"] % 4) * CANW + co["x"] // 2
            ev = xpar == 0
            idx_even[slot[ev]] = tgt[ev]
            idx_odd[slot[~ev]] = tgt[~ev]

            # scatter valid points into x16 columns
            pm = mk[kp]  # (n, 32)
            rows, cols_p = np.nonzero(pm)
            # rank of each valid point within its pillar
            cum = pm.cumsum(1) - 1
            rank = cum[rows, cols_p]
            col = colbase[slot[rows]] + rank
            feats = pf[kp[rows], cols_p, :]          # (npts, 7)
            base = xpar[rows] * 8
            x16t[col, base + 7] = 1.0
            for f in range(7):
                x16t[col, base + f] = feats[:, f]

        idx = np.empty((128, NSLOTS), np.int16)
        idx[0:64] = idx_even.astype(np.int16)
        idx[64:128] = idx_odd.astype(np.int16)

        rm = np.ones((128, 2), np.float32)
        if i == 0:
            rm[:, 0] = 0.0
        if i == NCORES - 1:
            rm[:, 1] = 0.0

        in_maps.append({
            "x16": np.ascontiguousarray(x16t.T).astype(NPBF16),
            "idxv": idx,
            "rmask": rm,
        })

    # shared weights
    w8 = np.concatenate([np.asarray(w_pfn, np.float32),
                         np.asarray(b_pfn, np.float32)[None, :]], 0)  # (8, 64)
    w16 = np.zeros((16, 128), np.float32)
    w16[0:8, 0:64] = w8
    w16[8:16, 64:128] = w8

    w1f = np.asarray(w1, np.float32)  # (128, 64, 3, 3)
    wp = np.zeros((128, 3 * 128), np.float32)
    wsg = np.zeros((64, 3 * 128), np.float32)
    for dy in range(3):
        wp[0:64, dy * 128 : (dy + 1) * 128] = w1f[:, :, dy, 0].T
        wp[64:128, dy * 128 : (dy + 1) * 128] = w1f[:, :, dy, 1].T
        wsg[:, dy * 128 : (dy + 1) * 128] = w1f[:, :, dy, 2].T

    w2f = np.asarray(w2, np.float32)
    wc2 = np.zeros((128, 9 * 128), np.float32)
    for k in range(9):
        dy, dx = k // 3, k % 3
        wc2[:, k * 128 : (k + 1) * 128] = w2f[:, :, dy, dx].T

    whd = np.ascontiguousarray(np.concatenate(
        [np.asarray(w_cls, np.float32)[:, :, 0, 0],
         np.asarray(w_box, np.float32)[:, :, 0, 0]], 0).T)
    bhd = np.concatenate([np.asarray(b_cls, np.float32),
                          np.asarray(b_box, np.float32)])[:, None].astype(np.float32)

    shared = {
        "w16": w16.astype(NPBF16),
        "wp": wp.astype(NPBF16),
        "ws": wsg.astype(NPBF16),
        "wc2": wc2.astype(NPBF16),
        "whd": whd.astype(NPBF16),
        "b1v": np.asarray(b1, np.float32)[:, None],
        "b2v": np.asarray(b2, np.float32)[:, None],
        "bhd": bhd,
    }
    for m in in_maps:
        m.update(shared)
    return in_maps, profile, tot


_CACHE = {}


def kernel(pillar_features, mask, coords, H=None, W=None,
           w_pfn=None, b_pfn=None, w1=None, b1=None, w2=None, b2=None,
           w_cls=None, b_cls=None, w_box=None, b_box=None):
    in_maps, profile, tot = _prep(pillar_features, mask, coords, w_pfn, b_pfn,
                                  w1, b1, w2, b2, w_cls, b_cls, w_box, b_box)
    key = (profile, tot)
    if _CACHE.get("key") != key:
        _CACHE["nc"] = _build_program(profile, tot)
        _CACHE["key"] = key
    nc = _CACHE["nc"]

    trace = os.environ.get("KERNEL_TRACE", "0") == "1"
    res = run_bass_kernel_spmd(nc, in_maps, core_ids=list(range(NCORES)),
                               trace=trace)
    if trace and res.exec_time_ns is not None:
        print(f"HW exec time: {res.exec_time_ns} ns")
        _CACHE["exec_time_ns"] = res.exec_time_ns
        _CACHE["res"] = res

    full = np.zeros((34, 256, 256), np.float32)
    for i in range(NCORES):
        full[:, 32 * i : 32 * i + 32, :] = res.results[i]["out"]
    return full[None]


# revision 6
# speedup vs baseline: 1.1766x; 1.1766x over previous
"""PointPillars Trainium2 kernel v2.

Pipeline per core (8 cores, 64-row horizontal bands of the 512x512 canvas):
  1. PFN: each matmul column holds one point of an even-x pillar (K rows
     0:8) AND one point of an odd-x pillar (rows 8:16) via block-diagonal
     weights (K=16, M=128 = even-pillar 64ch | odd-pillar 64ch).  Pillars
     are paired rank-wise by valid-point count within each row-quad;
     chunks of 32 pair-slots with data-dependent point depth (sorted,
     cross-core max, +1 all-zero column for built-in relu).  TensorE
     matmul -> PSUM, DVE tensor_reduce (max over points) -> emb, already
     in the x-parity-split partition layout the scatter needs.
  2. Scatter: gpsimd local_scatter writes embeddings straight into an
     x-parity-deinterleaved BEV canvas (partitions 0:64 = even x, 64:128 =
     odd x), zeroing untouched cells.  Host precomputes int16 indices.
  3. conv1 64->128 s2: x-parity pairing -> 6 matmul passes/chunk instead of
     9 (3 taps K=128 + 3 taps K=64), tap-major in groups of 4 psum banks.
  4. conv2 128->128 s1: 9 taps K=128 tap-major, groups of 4.
  5. head 1x1 -> 34ch, bias on ScalarE, DMA out.
"""

import os
import sys
from contextlib import ExitStack

sys.path.insert(0, "/opt/trn_rl_repo")

import numpy as np
import ml_dtypes

import concourse.bass as bass
import concourse.tile as tile
from concourse import bacc, mybir
from concourse.bass_utils import run_bass_kernel_spmd

BF16 = mybir.dt.bfloat16
F32 = mybir.dt.float32
I16 = mybir.dt.int16
NPBF16 = ml_dtypes.bfloat16

NCORES = 8
H = W = 512
P = 32
CE = 64
ROWS = 72          # rows per core band: 2 top halo + 64 owned + 6 bottom
NQ = ROWS // 4     # 18 row-quads
QSLOTS = 128       # pillar-pair slots per quad
NSLOTS = NQ * QSLOTS
SCH = 32           # pair-slots per PFN chunk
CANW = 257         # canvas cols: xe 0..255 data + col 256 zero
C1R = 34
OUTR = 32


# ----------------------------------------------------------------------------
# device program
# ----------------------------------------------------------------------------

def _build_program(profile, gtot):
    """profile: tuple of (q, c, D, grp, goff); chunk columns live in x16 group
    `grp` (SBUF partitions 32*grp..32*grp+16) at column offset goff.  gtot =
    column count of the widest group."""
    nc = bacc.Bacc(None, target_bir_lowering=False, debug=False)

    x16 = nc.dram_tensor("x16", [128, gtot], BF16, kind="ExternalInput")
    idxv = nc.dram_tensor("idxv", [128, NSLOTS], I16, kind="ExternalInput")
    w16 = nc.dram_tensor("w16", [128, 128], BF16, kind="ExternalInput")
    wp = nc.dram_tensor("wp", [128, 3 * 128], BF16, kind="ExternalInput")
    ws = nc.dram_tensor("ws", [64, 3 * 128], BF16, kind="ExternalInput")
    wc2 = nc.dram_tensor("wc2", [128, 9 * 128], BF16, kind="ExternalInput")
    whd = nc.dram_tensor("whd", [128, 34], BF16, kind="ExternalInput")
    b1v = nc.dram_tensor("b1v", [128, 1], F32, kind="ExternalInput")
    b2v = nc.dram_tensor("b2v", [128, 1], F32, kind="ExternalInput")
    bhd = nc.dram_tensor("bhd", [34, 1], F32, kind="ExternalInput")
    rmask = nc.dram_tensor("rmask", [128, 2], F32, kind="ExternalInput")
    out = nc.dram_tensor("out", [34, OUTR, 256], F32, kind="ExternalOutput")

    by_quad = {}
    for (q, c, d, grp, goff) in profile:
        by_quad.setdefault(q, []).append((c, d, grp, goff))

    with tile.TileContext(nc) as tc, ExitStack() as ctx:
        const = ctx.enter_context(tc.tile_pool(name="const", bufs=1))
        big = ctx.enter_context(tc.tile_pool(name="big", bufs=1))
        stg = ctx.enter_context(tc.tile_pool(name="staging", bufs=2))
        ps = ctx.enter_context(tc.tile_pool(name="psum", bufs=2, space="PSUM"))

        # ---- DMAs ordered for earliest compute start, split over 2 queues.
        # scalar queue leads with PFN weights; sync queue streams x16 slices.
        w16_sb = const.tile([128, 128], BF16)
        nc.scalar.dma_start(w16_sb[:], w16[:])
        x16_sb = big.tile([128, gtot], BF16)
        # slice boundaries: front-loaded so the PFN never waits on x16
        bnds = [0]
        for stop_q in (1, 4, 9):
            hi = 0
            for (q, c, d, grp, goff) in profile:
                if q <= stop_q:
                    hi = max(hi, goff + SCH * d)
            bnds.append(hi)
        bnds.append(gtot)
        bnds = sorted(set(bnds))
        for a, b in zip(bnds[:-1], bnds[1:]):
            if b > a:
                nc.sync.dma_start(x16_sb[:, a:b], x16[:, a:b])
        # scalar queue: scatter indices + conv weights in consumption order.
        idx_sb = big.tile([128, NSLOTS], I16)
        nc.scalar.dma_start(idx_sb[:], idxv[:])
        wp_sb = const.tile([128, 3 * 128], BF16)
        nc.scalar.dma_start(wp_sb[:], wp[:])
        ws_sb = const.tile([64, 3 * 128], BF16)
        nc.scalar.dma_start(ws_sb[:], ws[:])
        b1_sb = const.tile([128, 1], F32)
        nc.scalar.dma_start(b1_sb[:], b1v[:])
        rmask_sb = const.tile([128, 2], F32)
        nc.scalar.dma_start(rmask_sb[:], rmask[:])
        wc2_sb = const.tile([128, 9 * 128], BF16)
        nc.scalar.dma_start(wc2_sb[:], wc2[:])
        b2_sb = const.tile([128, 1], F32)
        nc.scalar.dma_start(b2_sb[:], b2v[:])
        whd_sb = const.tile([128, 34], BF16)
        nc.scalar.dma_start(whd_sb[:], whd[:])
        bhd_sb = const.tile([34, 1], F32)
        nc.scalar.dma_start(bhd_sb[:], bhd[:])

        emb2 = big.tile([128, NSLOTS], BF16)
        canvas2 = big.tile([128, ROWS, CANW], BF16)
        out1 = big.tile([128, C1R, 258], BF16)
        out2 = big.tile([128, OUTR, 256], BF16)

        nc.vector.memset(out1[:, :, 0:1], 0.0)
        nc.vector.memset(out1[:, :, 257:258], 0.0)

        # dummy local_scatter: forces the gpsimd ucode library swap to happen
        # during the initial DMA wait instead of stalling the first quad
        scr_i = const.tile([16, 2], I16)
        nc.vector.memset(scr_i[:], -1)
        scr_d = const.tile([16, 2], BF16)
        nc.vector.memset(scr_d[:], 0.0)
        scr_o = const.tile([16, 2], BF16)
        nc.gpsimd.local_scatter(scr_o[:], scr_d[:], scr_i[:],
                                channels=16, num_elems=2, num_idxs=2)

        def pfn_quad(q):
            for (c, d, grp, goff) in by_quad.get(q, []):
                cols = SCH * d
                p0 = 32 * grp
                pt = ps.tile([128, 1024], F32, tag="pfn", bufs=2, name=f"pt{q}_{c}")
                for a in range(0, cols, 512):
                    b = min(a + 512, cols)
                    nc.tensor.matmul(
                        pt[:, a:b],
                        lhsT=w16_sb[p0 : p0 + 16, :],
                        rhs=x16_sb[p0 : p0 + 16, goff + a : goff + b],
                        start=True, stop=True,
                    )
                s0 = q * QSLOTS + c * SCH
                nc.vector.tensor_reduce(
                    emb2[:, s0 : s0 + SCH],
                    pt[:, 0:cols].rearrange("p (s d) -> p s d", s=SCH),
                    axis=mybir.AxisListType.X,
                    op=mybir.AluOpType.max,
                )
            qb = q * QSLOTS
            nc.gpsimd.local_scatter(
                canvas2[:, 4 * q : 4 * q + 4, :],
                emb2[:, qb : qb + QSLOTS],
                idx_sb[:, qb : qb + QSLOTS],
                channels=128,
                num_elems=4 * CANW,
                num_idxs=QSLOTS,
            )

        def conv1_chunk(c):
            pc = ps.tile([128, 2, 256], F32, tag="mm", bufs=4, name=f"pc1_{c}")
            for ki in range(6):
                dy, paired = ki % 3, ki < 3
                r = 4 * c + dy
                if paired:
                    rhs = canvas2[:, r : r + 3 : 2, 0:256]
                    lhsT = wp_sb[:, dy * 128 : (dy + 1) * 128]
                else:
                    rhs = canvas2[0:64, r : r + 3 : 2, 1:257]
                    lhsT = ws_sb[:, dy * 128 : (dy + 1) * 128]
                nc.tensor.matmul(
                    pc[:], lhsT=lhsT, rhs=rhs, start=(ki == 0), stop=(ki == 5)
                )
            nc.scalar.activation(
                out1[:, 2 * c : 2 * c + 2, 1:257], pc[:],
                mybir.ActivationFunctionType.Relu, bias=b1_sb[:],
            )
            # zero conv1 halo rows that are conv2 SAME-padding at global edges
            # (on the scalar queue, right after the act that wrote the row)
            if c == 0:
                nc.scalar.mul(out1[:, 0:1, :], out1[:, 0:1, :], rmask_sb[:, 0:1])
            if c == C1R // 2 - 1:
                nc.scalar.mul(out1[:, 33:34, :], out1[:, 33:34, :], rmask_sb[:, 1:2])

        def conv2_chunk(c):
            o = 2 * c
            pc = ps.tile([128, 2, 256], F32, tag="mm", bufs=4, name=f"pc2_{c}")
            for k in range(9):
                dy, dx = k // 3, k % 3
                rhs = out1[:, o + dy : o + dy + 2, dx : dx + 256]
                nc.tensor.matmul(
                    pc[:], lhsT=wc2_sb[:, 128 * k : 128 * (k + 1)], rhs=rhs,
                    start=(k == 0), stop=(k == 8),
                )
            nc.scalar.activation(
                out2[:, o : o + 2, :], pc[:],
                mybir.ActivationFunctionType.Relu, bias=b2_sb[:],
            )

        def head_chunk(c):
            o = 2 * c
            ph = ps.tile([34, 2, 256], F32, tag="mm", bufs=4, name=f"ph_{c}")
            nc.tensor.matmul(
                ph[:], lhsT=whd_sb[:], rhs=out2[:, o : o + 2, :],
                start=True, stop=True,
            )
            hstage = stg.tile([34, 2, 256], F32, tag="hstage", bufs=4,
                              name=f"hs_{c}")
            nc.scalar.activation(
                hstage[:], ph[:], mybir.ActivationFunctionType.Identity,
                bias=bhd_sb[:],
            )
            eng = nc.sync if c % 2 == 0 else nc.scalar
            eng.dma_start(out[:, o : o + 2, :], hstage[:])

        # ---- fused quad pipeline with lag-2 so conv1 never waits on the
        # scatter of its own quad pair: PFN(s) | conv1(s-2) | conv2(s-3) |
        # head(s-4), plus drain steps.
        for s in range(NQ + 2):
            if s < NQ:
                pfn_quad(s)
            if 0 <= s - 2 < C1R // 2:
                conv1_chunk(s - 2)
            if 0 <= s - 3 < OUTR // 2:
                conv2_chunk(s - 3)
            if 0 <= s - 4 < OUTR // 2:
                head_chunk(s - 4)

    nc.compile()
    return nc


# ----------------------------------------------------------------------------
# host-side prep
# ----------------------------------------------------------------------------

def _prep(pillar_features, mask, coords, w_pfn, b_pfn,
          w1, b1, w2, b2, w_cls, b_cls, w_box, b_box):
    pf = np.asarray(pillar_features, np.float32)
    mk = np.asarray(mask, bool)
    xy = np.asarray(coords)
    x, y = xy[:, 0].astype(np.int64), xy[:, 1].astype(np.int64)

    valid = (x >= 0) & (x < W) & (y >= 0) & (y < H)
    lin = y * W + x
    # last-wins dedup among valid pillars (matches XLA scatter .set order)
    vidx = np.nonzero(valid)[0]
    order = vidx[np.argsort(lin[vidx], kind="stable")]
    ls = lin[order]
    is_last = np.ones(len(order), bool)
    if len(order) > 1:
        is_last[:-1] = ls[1:] != ls[:-1]
    keep = order[is_last]
    keep = keep[mk[keep].any(1)]  # empty pillars scatter zeros = canvas default

    kx, ky = x[keep], y[keep]
    kmask = mk[keep]
    vcnt = kmask.sum(1)

    # per (core, quad): pair even-x and odd-x pillars rank-wise by valid count
    cores = []
    for i in range(NCORES):
        y0 = 64 * i - 2
        sel = np.nonzero((ky >= y0) & (ky < y0 + ROWS))[0]
        rloc = (ky[sel] - y0).astype(np.int64)
        q = rloc // 4
        d = vcnt[sel]
        xpar = (kx[sel] % 2).astype(np.int64)
        slot = np.zeros(len(sel), np.int64)
        for t in range(NQ):
            for par in (0, 1):
                m = np.nonzero((q == t) & (xpar == par))[0]
                if len(m) > QSLOTS:
                    raise RuntimeError(f"quad overflow: {len(m)} > {QSLOTS}")
                o2 = m[np.argsort(-d[m], kind="stable")]
                slot[o2] = t * QSLOTS + np.arange(len(o2))
        cores.append({
            "keep": keep[sel], "rloc": rloc, "x": kx[sel],
            "depth": d, "slot": slot, "xpar": xpar,
        })

    # chunk profile: cross-core max point depth per (quad, chunk), +1 zero
    # col.  Chunks are spread over 4 column groups (SBUF partition blocks
    # 32g..32g+16) greedily balanced, in quad order so early quads sit at
    # low group offsets.
    prof = []
    goffs = [0, 0, 0]
    for t in range(NQ):
        for c in range(QSLOTS // SCH):
            cid = t * (QSLOTS // SCH) + c
            dmax = 0
            for co in cores:
                m = (co["slot"] // SCH) == cid
                if m.any():
                    dmax = max(dmax, int(co["depth"][m].max()))
            if dmax == 0:
                continue
            d = dmax + 1
            grp = int(np.argmin(goffs))
            prof.append((t, c, d, grp, goffs[grp]))
            goffs[grp] += SCH * d
    gtot = max(goffs)
    profile = tuple(prof)

    # per-slot (group, column base) from profile
    colgrp = np.zeros(NSLOTS, np.int64)
    colbase = np.full(NSLOTS, -1, np.int64)
    for (t, c, d, grp, goff) in prof:
        s0 = t * QSLOTS + c * SCH
        colgrp[s0 : s0 + SCH] = grp
        colbase[s0 : s0 + SCH] = goff + np.arange(SCH) * d

    in_maps = []
    for i, co in enumerate(cores):
        x16t = np.zeros((gtot, 128), np.float32)
        idx_even = np.full(NSLOTS, -1, np.int64)
        idx_odd = np.full(NSLOTS, -1, np.int64)

        kp, slot, xpar = co["keep"], co["slot"], co["xpar"]
        if len(kp):
            # target canvas index: (rloc%4)*CANW + xe
            tgt = (co["rloc"] % 4) * CANW + co["x"] // 2
            ev = xpar == 0
            idx_even[slot[ev]] = tgt[ev]
            idx_odd[slot[~ev]] = tgt[~ev]

            # scatter valid points into x16 columns
            pm = mk[kp]  # (n, 32)
            rows, cols_p = np.nonzero(pm)
            # rank of each valid point within its pillar
            cum = pm.cumsum(1) - 1
            rank = cum[rows, cols_p]
            col = colbase[slot[rows]] + rank
            feats = pf[kp[rows], cols_p, :]          # (npts, 7)
            base = 32 * colgrp[slot[rows]] + xpar[rows] * 8
            x16t[col, base + 7] = 1.0
            for f in range(7):
                x16t[col, base + f] = feats[:, f]

        idx = np.empty((128, NSLOTS), np.int16)
        idx[0:64] = idx_even.astype(np.int16)
        idx[64:128] = idx_odd.astype(np.int16)

        rm = np.ones((128, 2), np.float32)
        if i == 0:
            rm[:, 0] = 0.0
        if i == NCORES - 1:
            rm[:, 1] = 0.0

        in_maps.append({
            "x16": np.ascontiguousarray(x16t.T).astype(NPBF16),
            "idxv": idx,
            "rmask": rm,
        })

    # shared weights: PFN block-diag, replicated at the 4 group partition
    # offsets
    w8 = np.concatenate([np.asarray(w_pfn, np.float32),
                         np.asarray(b_pfn, np.float32)[None, :]], 0)  # (8, 64)
    w16 = np.zeros((128, 128), np.float32)
    for g in range(3):
        w16[32 * g : 32 * g + 8, 0:64] = w8
        w16[32 * g + 8 : 32 * g + 16, 64:128] = w8

    w1f = np.asarray(w1, np.float32)  # (128, 64, 3, 3)
    wp = np.zeros((128, 3 * 128), np.float32)
    wsg = np.zeros((64, 3 * 128), np.float32)
    for dy in range(3):
        wp[0:64, dy * 128 : (dy + 1) * 128] = w1f[:, :, dy, 0].T
        wp[64:128, dy * 128 : (dy + 1) * 128] = w1f[:, :, dy, 1].T
        wsg[:, dy * 128 : (dy + 1) * 128] = w1f[:, :, dy, 2].T

    w2f = np.asarray(w2, np.float32)
    wc2 = np.zeros((128, 9 * 128), np.float32)
    for k in range(9):
        dy, dx = k // 3, k % 3
        wc2[:, k * 128 : (k + 1) * 128] = w2f[:, :, dy, dx].T

    whd = np.ascontiguousarray(np.concatenate(
        [np.asarray(w_cls, np.float32)[:, :, 0, 0],
         np.asarray(w_box, np.float32)[:, :, 0, 0]], 0).T)
    bhd = np.concatenate([np.asarray(b_cls, np.float32),
                          np.asarray(b_box, np.float32)])[:, None].astype(np.float32)

    shared = {
        "w16": w16.astype(NPBF16),
        "wp": wp.astype(NPBF16),
        "ws": wsg.astype(NPBF16),
        "wc2": wc2.astype(NPBF16),
        "whd": whd.astype(NPBF16),
        "b1v": np.asarray(b1, np.float32)[:, None],
        "b2v": np.asarray(b2, np.float32)[:, None],
        "bhd": bhd,
    }
    for m in in_maps:
        m.update(shared)
    return in_maps, profile, gtot


_CACHE = {}


def kernel(pillar_features, mask, coords, H=None, W=None,
           w_pfn=None, b_pfn=None, w1=None, b1=None, w2=None, b2=None,
           w_cls=None, b_cls=None, w_box=None, b_box=None):
    in_maps, profile, gtot = _prep(pillar_features, mask, coords, w_pfn, b_pfn,
                                  w1, b1, w2, b2, w_cls, b_cls, w_box, b_box)
    key = (profile, gtot)
    if _CACHE.get("key") != key:
        _CACHE["nc"] = _build_program(profile, gtot)
        _CACHE["key"] = key
    nc = _CACHE["nc"]

    trace = os.environ.get("KERNEL_TRACE", "0") == "1"
    res = run_bass_kernel_spmd(nc, in_maps, core_ids=list(range(NCORES)),
                               trace=trace)
    if trace and res.exec_time_ns is not None:
        print(f"HW exec time: {res.exec_time_ns} ns")
        _CACHE["exec_time_ns"] = res.exec_time_ns
        _CACHE["res"] = res

    full = np.zeros((34, 256, 256), np.float32)
    for i in range(NCORES):
        full[:, 32 * i : 32 * i + 32, :] = res.results[i]["out"]
    return full[None]
